# revision 22
# baseline (speedup 1.0000x reference)
"""Trainium2 Bass kernel for nn_BiDecoder (gnn_message_passing).

out[e, c] = sum_s W_combine[c, s] * dot(ufeat[src[e]] @ Ps[s], ifeat[dst[e]])

Strategy (8 NeuronCores, SPMD single NEFF):
  - Edges sharded by src range; each core projects its ufeat shard on-device
    (PE matmul) into a single bf16 hu_cat[:, block, 2*D] table in SBUF.
  - Per-core edges bucketed by (src 128-block, dst chunk), dst-sorted within
    a bucket for HBM locality; each bucket is one prepare_only dma_gather of
    bf16 ifeat rows + trigger_dma (Pool does desc-gen only; transfer overlaps
    on 4 SWDGE queues).
  - Per tile: one-hot S built via bf16 broadcast matmul + ACT Square/Relu;
    U = S^T @ hu_cat (one bf16 matmul, both bases); per-basis dot via fused
    DVE tensor_tensor_reduce into r_buf; combine baked as scalar constants.
"""
import sys

sys.path.insert(0, "/opt/trn_rl_repo")
sys.path.insert(0, "/root/problem")

import os

import numpy as np

REGCNT = os.environ.get("K_REGCNT", "1") == "1"
PREP = os.environ.get("K_PREP", "1") == "1"
VBF16 = os.environ.get("K_VBF16", "1") == "1"

P = 128
D = 128
NB = 2
NCLS = 5
NCORES = 8

_COMPILED = {}
LAST_EXEC_NS = None
LAST_RESULTS = None
LAST_NC = None
LAST_INMAPS = None


def _tile_patch():
    from concourse import mybir
    from concourse import tile
    from concourse.vector_clock import ScopedClock

    def _drain_and_barrier(self, tick_clock, wait_clock):
        nc = self.nc
        drain_inst = nc.sync.drain()
        wait_clock.add_sem_waits(
            drain_inst.ins, ScopedClock({None: tick_clock.global_clock})
        )
        waits = list(drain_inst.ins.sync_info.on_wait)
        if len(waits) > 1:
            drain_inst.ins.sync_info = mybir.SyncInfo(on_wait=[], on_update=[])
            handles = {h.num: h for h in self.sems.allocated().values()}
            for w in waits:
                h = handles.get(w.id)
                assert h is not None, f"no sem handle for wait id {w.id}"
                assert w.wait_mode == "sem-ge-imm", w.wait_mode
                nc.sync.wait_ge(h, w.wait_value)
        nc.all_engine_barrier()
        assert self.sems is not None
        popped = nc._tile_sem_poison_stack.pop()
        assert popped is self._sem_poison
        nc.clear_and_free_semaphores(list(self.sems.allocated().values()))
        nc.all_engine_barrier()

    tile.TileContext._drain_and_barrier = _drain_and_barrier


class _Cfg:
    def __init__(self, users_pc, nipad, nchunk, t_bq, w):
        self.users_pc = users_pc
        self.nipad = nipad
        self.nchunk = nchunk
        self.t_bq = t_bq
        self.w = w
        self.blocks = users_pc // P
        self.chunk = nipad // nchunk
        assert self.chunk <= 32768
        self.ncalls = self.blocks * nchunk
        self.ni_call = t_bq * P
        self.nt = self.ncalls * t_bq
        self.slots = self.nt * P

    def key(self):
        return (self.users_pc, self.nipad, self.nchunk, self.t_bq, self.w)


def _build(nc, cfg):
    import concourse.mybir as mybir
    from concourse import tile
    from concourse import library_config

    f32, bf16, i16, i32 = (
        mybir.dt.float32,
        mybir.dt.bfloat16,
        mybir.dt.int16,
        mybir.dt.int32,
    )
    A = mybir.AluOpType
    AF = mybir.ActivationFunctionType

    vdt = bf16 if VBF16 else f32
    ufT = nc.dram_tensor("ufT", [P, cfg.users_pc], f32, kind="ExternalInput")
    ps = nc.dram_tensor("ps", [P, NB * D], f32, kind="ExternalInput")
    ifeats = [
        nc.dram_tensor(f"ifeat{q}", [cfg.chunk, D], vdt, kind="ExternalInput")
        for q in range(cfg.nchunk)
    ]
    dstidx = nc.dram_tensor("dstidx", [P, cfg.slots // 16], i16, kind="ExternalInput")
    srcrow = nc.dram_tensor("srcrow", [1, cfg.slots], bf16, kind="ExternalInput")
    gcnt = nc.dram_tensor("gcnt", [1, cfg.ncalls], i32, kind="ExternalInput")
    iotacol = nc.dram_tensor("iotacol", [P, 1], f32, kind="ExternalInput")
    onesrow = nc.dram_tensor("onesrow", [1, P], bf16, kind="ExternalInput")
    out = nc.dram_tensor("out", [cfg.slots, NCLS], f32, kind="ExternalOutput")

    mm = nc.tensor.matmul

    with tile.TileContext(nc) as tc:
        with (
            tc.tile_pool(name="tab", bufs=1) as tab,
            tc.tile_pool(name="cst", bufs=1) as cst,
            tc.tile_pool(name="io", bufs=3) as io,
            tc.tile_pool(name="gat", bufs=8) as gat,
            tc.tile_pool(name="wk", bufs=4) as wk,
            tc.tile_pool(name="pp", bufs=2, space="PSUM") as pp,
            tc.tile_pool(name="acc", bufs=1) as accp,
            tc.tile_pool(name="ob", bufs=2) as obp,
        ):
            nc.gpsimd.load_library(library_config.mlp)
            nreg = nc.gpsimd.register("n_idx").__enter__()
            nc.gpsimd.reg_mov(nreg, cfg.ni_call)
            dsems = [nc.alloc_semaphore(f"swdge{q}") for q in range(4)]

            iota_c = cst.tile([P, 1], f32)
            nc.sync.dma_start(out=iota_c[:], in_=iotacol[:])
            ones_r = cst.tile([1, P], bf16)
            nc.sync.dma_start(out=ones_r[:], in_=onesrow[:])
            ps_t = cst.tile([P, NB * D], f32)
            nc.sync.dma_start(out=ps_t[:], in_=ps[:])
            cnt_t = cst.tile([1, cfg.ncalls], i32)
            nc.sync.dma_start(out=cnt_t[:], in_=gcnt[:])

            hu = tab.tile([P, cfg.blocks, NB * D], bf16, tag="hu", name="hu")

            # ---- phase 0: project ufeat shard to bf16 hu (both bases) ----
            for b in range(cfg.blocks):
                uT = io.tile([P, P], f32, tag="uT")
                nc.sync.dma_start(out=uT[:], in_=ufT[:, b * P : (b + 1) * P])
                hps = pp.tile([P, NB * D], f32, tag="hups")
                mm(hps[:], lhsT=uT[:], rhs=ps_t[:], start=True, stop=True)
                nc.vector.tensor_copy(out=hu[:, b, :], in_=hps[:])

            # ---- phase 1 ----
            r_buf = accp.tile([P, cfg.nt, NB], f32)
            call = 0
            fires = [0, 0, 0, 0]
            idxcols = cfg.nchunk * cfg.ni_call // 16
            srowcols = cfg.nchunk * cfg.ni_call
            for b in range(cfg.blocks):
                idx_t = io.tile([P, idxcols], i16, tag="idx")
                nc.sync.dma_start(
                    out=idx_t[:], in_=dstidx[:, b * idxcols : (b + 1) * idxcols])
                srow = io.tile([1, srowcols], bf16, tag="srow")
                nc.sync.dma_start(
                    out=srow[:], in_=srcrow[:, b * srowcols : (b + 1) * srowcols])
                for q in range(cfg.nchunk):
                    qq = call % 4
                    if REGCNT:
                        nc.gpsimd.reg_load(nreg, cnt_t[0:1, call : call + 1])
                    v_g = gat.tile([P, cfg.t_bq, D], vdt, tag="vg")
                    if PREP:
                        nc.gpsimd.dma_gather(
                            out_ap=v_g[:],
                            in_ap=ifeats[q][:, :],
                            idxs_ap=idx_t[:, q * cfg.ni_call // 16 : (q + 1) * cfg.ni_call // 16],
                            num_idxs=cfg.ni_call,
                            num_idxs_reg=nreg,
                            elem_size=D,
                            queue_num=qq,
                            prepare_only=True,
                            sem=dsems[qq],
                        )
                        nc.gpsimd.trigger_dma(count=None, queue_num=qq)
                        fires[qq] += 1
                    else:
                        nc.gpsimd.dma_gather(
                            out_ap=v_g[:],
                            in_ap=ifeats[q][:, :],
                            idxs_ap=idx_t[:, q * cfg.ni_call // 16 : (q + 1) * cfg.ni_call // 16],
                            num_idxs=cfg.ni_call,
                            num_idxs_reg=nreg,
                            elem_size=D,
                            queue_num=qq,
                        )
                    so0 = q * cfg.ni_call
                    # one-hot S for the whole call: broadcast src ids via PE
                    # outer product (<=512 cols per PSUM bank), then a single
                    # DVE is_equal against the per-partition iota column.
                    bc = pp.tile([P, cfg.ni_call], f32, tag="bc")
                    for c0 in range(0, cfg.ni_call, 512):
                        cw = min(512, cfg.ni_call - c0)
                        mm(bc[:, c0 : c0 + cw], lhsT=ones_r[:1, :],
                           rhs=srow[:1, so0 + c0 : so0 + c0 + cw],
                           start=True, stop=True)
                    st = wk.tile([P, cfg.ni_call], bf16, tag="st")
                    nc.vector.tensor_scalar(
                        out=st[:], in0=bc[:], scalar1=iota_c[:, :1], scalar2=None,
                        op0=A.is_equal)
                    for t in range(cfg.t_bq):
                        gt = call * cfg.t_bq + t
                        u_ps = pp.tile([P, NB * D], f32, tag="ups")
                        mm(u_ps[:], lhsT=st[:, t * P : (t + 1) * P], rhs=hu[:, b, :],
                           start=True, stop=True)
                        u_bf = wk.tile([P, NB * D], bf16, tag="ubf")
                        nc.scalar.activation(u_bf[:], u_ps[:], AF.Copy)
                        for s in range(NB):
                            prod = wk.tile([P, D], bf16, tag=f"pr{s}")
                            nc.vector.scalar_tensor_tensor(
                                out=prod[:],
                                in0=u_bf[:, s * D : (s + 1) * D],
                                scalar=1.0,
                                in1=v_g[:, t, :],
                                op0=A.bypass,
                                op1=A.mult,
                                accum_out=r_buf[:, gt, s : s + 1],
                            )._wait_ge(dsems[qq], 16 * fires[qq])
                    call += 1

            # ---- phase 2: combine + store ----
            ch = 8
            while cfg.nt % ch:
                ch //= 2
            step = cfg.nt // ch
            outr = out.rearrange("(n p) c -> p n c", p=P)
            for k in range(ch):
                sl = slice(k * step, (k + 1) * step)
                ob = obp.tile([P, step, NCLS], f32, tag="ob")
                t0 = obp.tile([P, step], f32, tag="t0")
                t1 = obp.tile([P, step], f32, tag="t1")
                for c in range(NCLS):
                    nc.vector.tensor_scalar_mul(t0[:], r_buf[:, sl, 0], float(cfg.w[c][0]))
                    nc.vector.tensor_scalar_mul(t1[:], r_buf[:, sl, 1], float(cfg.w[c][1]))
                    nc.vector.tensor_tensor(out=ob[:, :, c], in0=t0[:], in1=t1[:], op=A.add)
                nc.sync.dma_start(out=outr[:, sl, :], in_=ob[:])
    return nc


def _host_prep_core(src_l, dst, cfg):
    b = (src_l >> 7).astype(np.int64)
    q = dst // cfg.chunk
    key = b * cfg.nchunk + q
    # secondary sort by dst for HBM locality within each gather call
    order = np.lexsort((dst, key))
    srt = order
    ks = key[srt]
    counts = np.bincount(ks, minlength=cfg.ncalls)
    if counts.max() > cfg.ni_call:
        raise OverflowError(f"bucket overflow {counts.max()} > {cfg.ni_call}")
    slot_edge = np.full(cfg.slots, -1, dtype=np.int64)
    src_rel = np.zeros(cfg.slots, dtype=np.float32)
    dst_rel = np.full(cfg.slots, -1, dtype=np.int16)
    # bucket starts in sorted array
    starts = np.zeros(cfg.ncalls + 1, dtype=np.int64)
    np.cumsum(counts, out=starts[1:])
    # slot position for each sorted edge
    arange = np.arange(len(srt), dtype=np.int64)
    slot_of_sorted = (ks * cfg.ni_call) + (arange - starts[ks])
    slot_edge[slot_of_sorted] = srt
    src_rel[slot_of_sorted] = (src_l[srt] & 127).astype(np.float32)
    dst_rel[slot_of_sorted] = (dst[srt] % cfg.chunk).astype(np.int16)
    # per-call valid-index counts; keep >=16 so every DMA engine gets a
    # descriptor (pad with dummy row-0 gathers when a bucket is tiny)
    gcnt = counts.astype(np.int32)
    for c in np.nonzero(gcnt < 16)[0]:
        dst_rel[c * cfg.ni_call + gcnt[c] : c * cfg.ni_call + 16] = 0
        gcnt[c] = 16
    if not REGCNT:
        dst_rel[dst_rel < 0] = 0
        gcnt[:] = cfg.ni_call
    w = dst_rel.reshape(cfg.ncalls, cfg.ni_call // 16, 16).transpose(0, 2, 1)
    wrapped = w.reshape(cfg.ncalls, 16, cfg.ni_call // 16)
    wrapped = np.concatenate(list(wrapped), axis=1)  # [16, slots/16]
    dstidx = np.tile(wrapped, (8, 1))
    return {
        "dstidx": np.ascontiguousarray(dstidx),
        "srcrow": np.ascontiguousarray(src_rel[None, :]),
        "gcnt": gcnt[None, :],
        "slot_edge": slot_edge,
    }


def _to_bf16(a):
    import ml_dtypes

    return a.astype(ml_dtypes.bfloat16)


def kernel(ufeat, ifeat, Ps, W_combine, src, dst, _trace=False):
    global LAST_EXEC_NS, LAST_RESULTS
    _tile_patch()
    import concourse.bacc as bacc
    from concourse.bass_utils import run_bass_kernel_spmd

    ufeat = np.asarray(ufeat, dtype=np.float32)
    ifeat = np.asarray(ifeat, dtype=np.float32)
    Ps = np.asarray(Ps, dtype=np.float32)
    W = np.asarray(W_combine, dtype=np.float32)
    src = np.asarray(src).astype(np.int64)
    dst = np.asarray(dst).astype(np.int64)
    E = src.shape[0]
    NU = ufeat.shape[0]
    NI = ifeat.shape[0]

    users_pc = ((NU + NCORES * P - 1) // (NCORES * P)) * P
    nupad = users_pc * NCORES
    nchunk = 4
    nipad = ((NI + nchunk * P - 1) // (nchunk * P)) * (nchunk * P)

    ufeat_p = np.zeros((nupad, D), np.float32)
    ufeat_p[:NU] = ufeat
    ifeat_p = np.zeros((nipad, D), np.float32)
    ifeat_p[:NI] = ifeat

    core_of = src // users_pc
    wtup = tuple(tuple(float(x) for x in r) for r in W)

    # choose t_bq from actual bucket maxima (uniform across cores for SPMD)
    t_bq = 5
    while True:
        cfg = _Cfg(users_pc, nipad, nchunk, t_bq, wtup)
        try:
            preps = []
            core_ids_list = []
            for c in range(NCORES):
                m = core_of == c
                eids = np.nonzero(m)[0]
                preps.append(
                    _host_prep_core(src[eids] - c * users_pc, dst[eids], cfg))
                core_ids_list.append(eids)
            break
        except OverflowError:
            t_bq += 1

    key = cfg.key()
    if key not in _COMPILED:
        nc = bacc.Bacc(num_swdge_queues=4)
        _build(nc, cfg)
        nc.compile()
        _COMPILED[key] = nc
    nc = _COMPILED[key]

    iotac = np.arange(P, dtype=np.float32)[:, None]
    ones = _to_bf16(np.ones((1, P), np.float32))
    psin = np.concatenate([Ps[0], Ps[1]], axis=1).astype(np.float32)
    ifeat_bf = _to_bf16(ifeat_p) if VBF16 else ifeat_p

    in_maps = []
    for c in range(NCORES):
        im = {
            "ufT": np.ascontiguousarray(ufeat_p[c * users_pc : (c + 1) * users_pc].T),
            "ps": psin,
            "dstidx": preps[c]["dstidx"],
            "srcrow": _to_bf16(preps[c]["srcrow"]),
            "gcnt": np.ascontiguousarray(preps[c]["gcnt"]),
            "iotacol": iotac,
            "onesrow": ones,
        }
        for q in range(cfg.nchunk):
            im[f"ifeat{q}"] = ifeat_bf[q * cfg.chunk : (q + 1) * cfg.chunk]
        in_maps.append(im)

    global LAST_NC, LAST_INMAPS
    LAST_NC = nc
    LAST_INMAPS = in_maps
    res = run_bass_kernel_spmd(nc, in_maps, core_ids=list(range(NCORES)),
                               trace=_trace)
    LAST_EXEC_NS = res.exec_time_ns
    LAST_RESULTS = res

    outfull = np.zeros((E, NCLS), np.float32)
    for c in range(NCORES):
        got = res.results[c]["out"]
        se = preps[c]["slot_edge"]
        v = se >= 0
        outfull[core_ids_list[c][se[v]]] = got[v]
    return outfull


# revision 24
# speedup vs baseline: 1.7813x; 1.7813x over previous
"""Trainium2 Bass kernel for nn_BiDecoder (gnn_message_passing).

out[e, c] = sum_s W_combine[c, s] * dot(ufeat[src[e]] @ Ps[s], ifeat[dst[e]])

Strategy (8 NeuronCores, SPMD single NEFF):
  - Edges sharded by src range; each core projects its ufeat shard on-device
    (PE matmul) into a single bf16 hu_cat[:, block, 2*D] table in SBUF.
  - Per-core edges bucketed by (src 128-block, dst chunk), dst-sorted within
    a bucket for HBM locality; each bucket is one prepare_only dma_gather of
    bf16 ifeat rows + trigger_dma (Pool does desc-gen only; transfer overlaps
    on 4 SWDGE queues).
  - Per tile: one-hot S built via bf16 broadcast matmul + ACT Square/Relu;
    U = S^T @ hu_cat (one bf16 matmul, both bases); per-basis dot via fused
    DVE tensor_tensor_reduce into r_buf; combine baked as scalar constants.
"""
import sys

sys.path.insert(0, "/opt/trn_rl_repo")
sys.path.insert(0, "/root/problem")

import os

import numpy as np

REGCNT = os.environ.get("K_REGCNT", "0") == "1"
PREP = os.environ.get("K_PREP", "0") == "1"
VBF16 = os.environ.get("K_VBF16", "1") == "1"

P = 128
D = 128
NB = 2
NCLS = 5
NCORES = 8

_COMPILED = {}
LAST_EXEC_NS = None
LAST_RESULTS = None
LAST_NC = None
LAST_INMAPS = None


def _tile_patch():
    from concourse import mybir
    from concourse import tile
    from concourse.vector_clock import ScopedClock

    def _drain_and_barrier(self, tick_clock, wait_clock):
        nc = self.nc
        drain_inst = nc.sync.drain()
        wait_clock.add_sem_waits(
            drain_inst.ins, ScopedClock({None: tick_clock.global_clock})
        )
        waits = list(drain_inst.ins.sync_info.on_wait)
        if len(waits) > 1:
            drain_inst.ins.sync_info = mybir.SyncInfo(on_wait=[], on_update=[])
            handles = {h.num: h for h in self.sems.allocated().values()}
            for w in waits:
                h = handles.get(w.id)
                assert h is not None, f"no sem handle for wait id {w.id}"
                assert w.wait_mode == "sem-ge-imm", w.wait_mode
                nc.sync.wait_ge(h, w.wait_value)
        nc.all_engine_barrier()
        assert self.sems is not None
        popped = nc._tile_sem_poison_stack.pop()
        assert popped is self._sem_poison
        nc.clear_and_free_semaphores(list(self.sems.allocated().values()))
        nc.all_engine_barrier()

    tile.TileContext._drain_and_barrier = _drain_and_barrier


class _Cfg:
    def __init__(self, users_pc, nipad, nchunk, t_bq, w):
        self.users_pc = users_pc
        self.nipad = nipad
        self.nchunk = nchunk
        self.t_bq = t_bq
        self.w = w
        self.blocks = users_pc // P
        self.chunk = nipad // nchunk
        assert self.chunk <= 32768
        self.ncalls = self.blocks * nchunk
        self.ni_call = t_bq * P
        self.nt = self.ncalls * t_bq
        self.slots = self.nt * P

    def key(self):
        return (self.users_pc, self.nipad, self.nchunk, self.t_bq, self.w)


def _build(nc, cfg):
    import concourse.mybir as mybir
    from concourse import tile
    from concourse import library_config

    f32, bf16, i16, i32 = (
        mybir.dt.float32,
        mybir.dt.bfloat16,
        mybir.dt.int16,
        mybir.dt.int32,
    )
    A = mybir.AluOpType
    AF = mybir.ActivationFunctionType

    vdt = bf16 if VBF16 else f32
    ufT = nc.dram_tensor("ufT", [P, cfg.users_pc], f32, kind="ExternalInput")
    ps = nc.dram_tensor("ps", [P, NB * D], f32, kind="ExternalInput")
    ifeats = [
        nc.dram_tensor(f"ifeat{q}", [cfg.chunk, D], vdt, kind="ExternalInput")
        for q in range(cfg.nchunk)
    ]
    dstidx = nc.dram_tensor("dstidx", [P, cfg.slots // 16], i16, kind="ExternalInput")
    srcrow = nc.dram_tensor("srcrow", [1, cfg.slots], bf16, kind="ExternalInput")
    gcnt = nc.dram_tensor("gcnt", [1, cfg.ncalls], i32, kind="ExternalInput")
    iotacol = nc.dram_tensor("iotacol", [P, 1], f32, kind="ExternalInput")
    onesrow = nc.dram_tensor("onesrow", [1, P], bf16, kind="ExternalInput")
    out = nc.dram_tensor("out", [cfg.slots, NCLS], f32, kind="ExternalOutput")

    mm = nc.tensor.matmul

    with tile.TileContext(nc) as tc:
        with (
            tc.tile_pool(name="tab", bufs=1) as tab,
            tc.tile_pool(name="cst", bufs=1) as cst,
            tc.tile_pool(name="io", bufs=3) as io,
            tc.tile_pool(name="gat", bufs=16) as gat,
            tc.tile_pool(name="wk", bufs=4) as wk,
            tc.tile_pool(name="pp", bufs=2, space="PSUM") as pp,
            tc.tile_pool(name="acc", bufs=1) as accp,
            tc.tile_pool(name="ob", bufs=2) as obp,
        ):
            nc.gpsimd.load_library(library_config.mlp)
            nreg = nc.gpsimd.register("n_idx").__enter__()
            nc.gpsimd.reg_mov(nreg, cfg.ni_call)
            dsems = [nc.alloc_semaphore(f"swdge{q}") for q in range(4)]

            iota_c = cst.tile([P, 1], f32)
            nc.sync.dma_start(out=iota_c[:], in_=iotacol[:])
            ones_r = cst.tile([1, P], bf16)
            nc.sync.dma_start(out=ones_r[:], in_=onesrow[:])
            ps_t = cst.tile([P, NB * D], f32)
            nc.sync.dma_start(out=ps_t[:], in_=ps[:])
            cnt_t = cst.tile([1, cfg.ncalls], i32)
            nc.sync.dma_start(out=cnt_t[:], in_=gcnt[:])

            hu = tab.tile([P, cfg.blocks, NB * D], bf16, tag="hu", name="hu")

            # ---- phase 0: project ufeat shard to bf16 hu (both bases) ----
            for b in range(cfg.blocks):
                uT = io.tile([P, P], f32, tag="uT")
                nc.sync.dma_start(out=uT[:], in_=ufT[:, b * P : (b + 1) * P])
                hps = pp.tile([P, NB * D], f32, tag="hups")
                mm(hps[:], lhsT=uT[:], rhs=ps_t[:], start=True, stop=True)
                nc.vector.tensor_copy(out=hu[:, b, :], in_=hps[:])

            # ---- phase 1 ----
            r_buf = accp.tile([P, cfg.nt, NB], f32)
            call = 0
            fires = [0, 0, 0, 0]
            idxcols = cfg.nchunk * cfg.ni_call // 16
            srowcols = cfg.nchunk * cfg.ni_call
            for b in range(cfg.blocks):
                idx_t = io.tile([P, idxcols], i16, tag="idx")
                nc.sync.dma_start(
                    out=idx_t[:], in_=dstidx[:, b * idxcols : (b + 1) * idxcols])
                srow = io.tile([1, srowcols], bf16, tag="srow")
                nc.sync.dma_start(
                    out=srow[:], in_=srcrow[:, b * srowcols : (b + 1) * srowcols])
                for q in range(cfg.nchunk):
                    qq = call % 4
                    if REGCNT:
                        nc.gpsimd.reg_load(nreg, cnt_t[0:1, call : call + 1])
                    v_g = gat.tile([P, cfg.t_bq, D], vdt, tag="vg")
                    if PREP:
                        nc.gpsimd.dma_gather(
                            out_ap=v_g[:],
                            in_ap=ifeats[q][:, :],
                            idxs_ap=idx_t[:, q * cfg.ni_call // 16 : (q + 1) * cfg.ni_call // 16],
                            num_idxs=cfg.ni_call,
                            num_idxs_reg=nreg,
                            elem_size=D,
                            queue_num=qq,
                            prepare_only=True,
                            sem=dsems[qq],
                        )
                        nc.gpsimd.trigger_dma(count=None, queue_num=qq)
                        fires[qq] += 1
                    else:
                        nc.gpsimd.dma_gather(
                            out_ap=v_g[:],
                            in_ap=ifeats[q][:, :],
                            idxs_ap=idx_t[:, q * cfg.ni_call // 16 : (q + 1) * cfg.ni_call // 16],
                            num_idxs=cfg.ni_call,
                            num_idxs_reg=nreg,
                            elem_size=D,
                            queue_num=qq,
                        )
                    so0 = q * cfg.ni_call
                    # one-hot S for the whole call: broadcast src ids via PE
                    # outer product (<=512 cols per PSUM bank), then a single
                    # DVE is_equal against the per-partition iota column.
                    bc = pp.tile([P, cfg.ni_call], f32, tag="bc")
                    for c0 in range(0, cfg.ni_call, 512):
                        cw = min(512, cfg.ni_call - c0)
                        mm(bc[:, c0 : c0 + cw], lhsT=ones_r[:1, :],
                           rhs=srow[:1, so0 + c0 : so0 + c0 + cw],
                           start=True, stop=True)
                    st = wk.tile([P, cfg.ni_call], bf16, tag="st")
                    nc.vector.tensor_scalar(
                        out=st[:], in0=bc[:], scalar1=iota_c[:, :1], scalar2=None,
                        op0=A.is_equal)
                    for t in range(cfg.t_bq):
                        gt = call * cfg.t_bq + t
                        u_ps = pp.tile([P, NB * D], f32, tag="ups")
                        mm(u_ps[:], lhsT=st[:, t * P : (t + 1) * P], rhs=hu[:, b, :],
                           start=True, stop=True)
                        u_bf = wk.tile([P, NB * D], bf16, tag="ubf")
                        nc.scalar.activation(u_bf[:], u_ps[:], AF.Copy)
                        for s in range(NB):
                            prod = wk.tile([P, D], bf16, tag=f"pr{s}")
                            nc.vector.scalar_tensor_tensor(
                                out=prod[:],
                                in0=u_bf[:, s * D : (s + 1) * D],
                                scalar=1.0,
                                in1=v_g[:, t, :],
                                op0=A.bypass,
                                op1=A.mult,
                                accum_out=r_buf[:, gt, s : s + 1],
                            )._wait_ge(dsems[qq], 16 * fires[qq])
                    call += 1

            # ---- phase 2: combine + store ----
            ch = 8
            while cfg.nt % ch:
                ch //= 2
            step = cfg.nt // ch
            outr = out.rearrange("(n p) c -> p n c", p=P)
            for k in range(ch):
                sl = slice(k * step, (k + 1) * step)
                ob = obp.tile([P, step, NCLS], f32, tag="ob")
                t0 = obp.tile([P, step], f32, tag="t0")
                t1 = obp.tile([P, step], f32, tag="t1")
                for c in range(NCLS):
                    nc.vector.tensor_scalar_mul(t0[:], r_buf[:, sl, 0], float(cfg.w[c][0]))
                    nc.vector.tensor_scalar_mul(t1[:], r_buf[:, sl, 1], float(cfg.w[c][1]))
                    nc.vector.tensor_tensor(out=ob[:, :, c], in0=t0[:], in1=t1[:], op=A.add)
                nc.sync.dma_start(out=outr[:, sl, :], in_=ob[:])
    return nc


def _host_prep_core(src_l, dst, cfg):
    b = (src_l >> 7).astype(np.int64)
    q = dst // cfg.chunk
    key = b * cfg.nchunk + q
    # secondary sort by dst for HBM locality within each gather call
    order = np.lexsort((dst, key))
    srt = order
    ks = key[srt]
    counts = np.bincount(ks, minlength=cfg.ncalls)
    if counts.max() > cfg.ni_call:
        raise OverflowError(f"bucket overflow {counts.max()} > {cfg.ni_call}")
    slot_edge = np.full(cfg.slots, -1, dtype=np.int64)
    src_rel = np.zeros(cfg.slots, dtype=np.float32)
    dst_rel = np.full(cfg.slots, -1, dtype=np.int16)
    # bucket starts in sorted array
    starts = np.zeros(cfg.ncalls + 1, dtype=np.int64)
    np.cumsum(counts, out=starts[1:])
    # slot position for each sorted edge
    arange = np.arange(len(srt), dtype=np.int64)
    slot_of_sorted = (ks * cfg.ni_call) + (arange - starts[ks])
    slot_edge[slot_of_sorted] = srt
    src_rel[slot_of_sorted] = (src_l[srt] & 127).astype(np.float32)
    dst_rel[slot_of_sorted] = (dst[srt] % cfg.chunk).astype(np.int16)
    # per-call valid-index counts; keep >=16 so every DMA engine gets a
    # descriptor (pad with dummy row-0 gathers when a bucket is tiny)
    gcnt = counts.astype(np.int32)
    for c in np.nonzero(gcnt < 16)[0]:
        dst_rel[c * cfg.ni_call + gcnt[c] : c * cfg.ni_call + 16] = 0
        gcnt[c] = 16
    if not REGCNT:
        dst_rel[dst_rel < 0] = 0
        gcnt[:] = cfg.ni_call
    w = dst_rel.reshape(cfg.ncalls, cfg.ni_call // 16, 16).transpose(0, 2, 1)
    wrapped = w.reshape(cfg.ncalls, 16, cfg.ni_call // 16)
    wrapped = np.concatenate(list(wrapped), axis=1)  # [16, slots/16]
    dstidx = np.tile(wrapped, (8, 1))
    return {
        "dstidx": np.ascontiguousarray(dstidx),
        "srcrow": np.ascontiguousarray(src_rel[None, :]),
        "gcnt": gcnt[None, :],
        "slot_edge": slot_edge,
    }


def _to_bf16(a):
    import ml_dtypes

    return a.astype(ml_dtypes.bfloat16)


def kernel(ufeat, ifeat, Ps, W_combine, src, dst, _trace=False):
    global LAST_EXEC_NS, LAST_RESULTS
    _tile_patch()
    import concourse.bacc as bacc
    from concourse.bass_utils import run_bass_kernel_spmd

    ufeat = np.asarray(ufeat, dtype=np.float32)
    ifeat = np.asarray(ifeat, dtype=np.float32)
    Ps = np.asarray(Ps, dtype=np.float32)
    W = np.asarray(W_combine, dtype=np.float32)
    src = np.asarray(src).astype(np.int64)
    dst = np.asarray(dst).astype(np.int64)
    E = src.shape[0]
    NU = ufeat.shape[0]
    NI = ifeat.shape[0]

    users_pc = ((NU + NCORES * P - 1) // (NCORES * P)) * P
    nupad = users_pc * NCORES
    nchunk = 4
    nipad = ((NI + nchunk * P - 1) // (nchunk * P)) * (nchunk * P)

    ufeat_p = np.zeros((nupad, D), np.float32)
    ufeat_p[:NU] = ufeat
    ifeat_p = np.zeros((nipad, D), np.float32)
    ifeat_p[:NI] = ifeat

    core_of = src // users_pc
    wtup = tuple(tuple(float(x) for x in r) for r in W)

    # choose t_bq from actual bucket maxima (uniform across cores for SPMD)
    t_bq = 5
    while True:
        cfg = _Cfg(users_pc, nipad, nchunk, t_bq, wtup)
        try:
            preps = []
            core_ids_list = []
            for c in range(NCORES):
                m = core_of == c
                eids = np.nonzero(m)[0]
                preps.append(
                    _host_prep_core(src[eids] - c * users_pc, dst[eids], cfg))
                core_ids_list.append(eids)
            break
        except OverflowError:
            t_bq += 1

    key = cfg.key()
    if key not in _COMPILED:
        nc = bacc.Bacc(num_swdge_queues=4)
        _build(nc, cfg)
        nc.compile()
        _COMPILED[key] = nc
    nc = _COMPILED[key]

    iotac = np.arange(P, dtype=np.float32)[:, None]
    ones = _to_bf16(np.ones((1, P), np.float32))
    psin = np.concatenate([Ps[0], Ps[1]], axis=1).astype(np.float32)
    ifeat_bf = _to_bf16(ifeat_p) if VBF16 else ifeat_p

    in_maps = []
    for c in range(NCORES):
        im = {
            "ufT": np.ascontiguousarray(ufeat_p[c * users_pc : (c + 1) * users_pc].T),
            "ps": psin,
            "dstidx": preps[c]["dstidx"],
            "srcrow": _to_bf16(preps[c]["srcrow"]),
            "gcnt": np.ascontiguousarray(preps[c]["gcnt"]),
            "iotacol": iotac,
            "onesrow": ones,
        }
        for q in range(cfg.nchunk):
            im[f"ifeat{q}"] = ifeat_bf[q * cfg.chunk : (q + 1) * cfg.chunk]
        in_maps.append(im)

    global LAST_NC, LAST_INMAPS
    LAST_NC = nc
    LAST_INMAPS = in_maps
    res = run_bass_kernel_spmd(nc, in_maps, core_ids=list(range(NCORES)),
                               trace=_trace)
    LAST_EXEC_NS = res.exec_time_ns
    LAST_RESULTS = res

    outfull = np.zeros((E, NCLS), np.float32)
    for c in range(NCORES):
        got = res.results[c]["out"]
        se = preps[c]["slot_edge"]
        v = se >= 0
        outfull[core_ids_list[c][se[v]]] = got[v]
    return outfull
